# revision 24
# baseline (speedup 1.0000x reference)
"""Trainium2 Bass kernel: Encoder_HieStackedCorr (UnCorrVmat_Detail, t_method='uncorr').

Math (per batch b):
    W1 = wn(U1_v, U1_g); W2 = wn(U2_v, U2_g)
    R = relu(V @ W1.T + b1)          [N, LR]
    L = relu(V @ W2.T + b2)          [N, LR]
    UnCorr = L @ R.T                 [N, N]
    d[n] = UnCorr[n, n] = sum_l L[n,l] R[n,l]
    dr = 1/sqrt(d + eps)
    A = 1 + I - dr dr^T * UnCorr
    v = mean_n (A @ V) = (1/N) * s @ V  where s[m] = N + 1 - dr[m] * (t . R[m,:]),
                                              t = sum_n dr[n] L[n,:]
    feat = v @ W_lin.T + b_lin ; out = batchnorm(feat)   (training-mode stats)

The N x N matrix is never materialized: mean-pooling commutes with the matmul,
collapsing the O(B N^2 (LR+D)) reference into O(B N D LR) work.

Kernel structure (v2):
  - W1 || W2 stacked into one [128, 128] lhsT: a single matmul pair per
    512-block produces both R (partitions 0-63) and L (64-127); one fused
    relu+bias on the whole [128, 512] PSUM tile.
  - Diag via ones[64,64] lhsT: d arrives already broadcast over 64
    partitions ([64, 512] PSUM), so rsqrt runs wide (ACT sqrt + DVE
    reciprocal_approx_fast) and no separate broadcast matmul is needed.
  - t via chained tensor_tensor_reduce (mult+add accumulate across blocks).
  - dr is folded into R (Rd = R * drrep on GpSimd), so the u-matmul yields
    c = dr*u directly and s = (N+1-c)/N comes from one ACT affine per block.
  - s scattered to column layout via a DRAM bounce, then 16 accumulating
    matmuls against natural-layout V give v_mean.

Sharding: data-parallel over batch, 4 batches per core on 8 cores.  The tiny
[32,256] linear + batchnorm epilogue (cross-core batch stats) runs on host.
"""

import os
import sys

import numpy as np

for _p in ("/opt/trn_rl_repo", "/root/.axon_site/_ro/trn_rl_repo"):
    if os.path.isdir(_p) and _p not in sys.path:
        sys.path.insert(0, _p)
        break

import ml_dtypes  # noqa: E402
import concourse.bass as bass  # noqa: E402
import concourse.bacc as bacc  # noqa: E402
import concourse.mybir as mybir  # noqa: E402
import concourse.tile as tile  # noqa: E402
from concourse.bass_utils import run_bass_kernel_spmd  # noqa: E402


def _ensure_ntff_hook():
    """Shim the missing ``antenv.axon_hooks`` registry so trace=True works."""
    import types

    try:
        from antenv.axon_hooks import get_axon_ntff_profile_hook  # noqa: F401
        return
    except ImportError:
        pass
    try:
        from trn_agent_boot.trn_boot import _ntff_profile_via_ctypes
        hook = _ntff_profile_via_ctypes("/opt/axon/libaxon_pjrt.so")
    except Exception:
        hook = None
    mod = types.ModuleType("antenv.axon_hooks")
    mod._hook = hook
    mod.get_axon_ntff_profile_hook = lambda: mod._hook
    mod.set_axon_ntff_profile_hook = lambda h: setattr(mod, "_hook", h)
    sys.modules["antenv.axon_hooks"] = mod


_ensure_ntff_hook()

# Problem constants (hardcoded; see module docstring).
B, N, D, LR, EMB = 32, 2048, 256, 64, 256
NCORES = 8
B_LOC = B // NCORES          # 4 batches per core
NT_B = N // 128              # 16 row-tiles per batch
NBLK = N // 512              # 4 512-col blocks per batch
VIN_W = 2 * N + NT_B * D     # per-batch packed width: vt (2*N) + v-nat (16*D)
EPS_DIAG = 1e-6
EPS_BN = 1e-5

F32 = mybir.dt.float32
BF16 = mybir.dt.bfloat16

CONFIG = dict(dt="bf16", trace=False)

_CACHE = {}


def _build(cfg):
    nc = bacc.Bacc("TRN2", target_bir_lowering=False, debug=False)

    vin_d = nc.dram_tensor("vin", [128, B_LOC, VIN_W], BF16, kind="ExternalInput").ap()
    w12_d = nc.dram_tensor("w12", [128, 2, 128], BF16, kind="ExternalInput").ap()
    b12_d = nc.dram_tensor("b12", [128, 1], F32, kind="ExternalInput").ap()
    out_d = nc.dram_tensor("vmean", [2, B_LOC * 512], F32, kind="ExternalOutput").ap()

    AF = mybir.ActivationFunctionType
    OP = mybir.AluOpType

    with tile.TileContext(nc) as tc:
        with (
            tc.tile_pool(name="const", bufs=1) as cpool,
            tc.tile_pool(name="vin", bufs=B_LOC) as vpool,
            tc.tile_pool(name="lr", bufs=3) as lrpool,
            tc.tile_pool(name="rd", bufs=3) as rdpool,
            tc.tile_pool(name="blk", bufs=3) as bpool,
            tc.tile_pool(name="sq", bufs=3) as sqpool,
            tc.tile_pool(name="dr", bufs=3) as drpool,
            tc.tile_pool(name="tsm", bufs=12) as tpool,
            tc.tile_pool(name="srow", bufs=3) as srpool,
            tc.tile_pool(name="scol", bufs=3) as scpool,
            tc.tile_pool(name="ps_lr", bufs=2, space="PSUM") as ps_lr,
            tc.tile_pool(name="ps_d", bufs=2, space="PSUM") as ps_d,
            tc.tile_pool(name="ps_u", bufs=2, space="PSUM") as ps_u,
            tc.tile_pool(name="ps_v", bufs=2, space="PSUM") as ps_v,
            tc.tile_pool(name="dram", bufs=4, space="DRAM") as dpool,
        ):
            # ---- constants / weights ----
            w12_sb = cpool.tile([128, 2 * 128], BF16)
            nc.sync.dma_start(
                w12_sb[:].rearrange("p (c m) -> p c m", c=2), w12_d[:]
            )
            b12_sb = cpool.tile([128, 1], F32)
            nc.sync.dma_start(b12_sb[:], b12_d[:])
            ones64 = cpool.tile([64, 64], BF16)
            nc.vector.memset(ones64[:], 1.0)
            eps64 = cpool.tile([64, 1], F32)
            nc.vector.memset(eps64[:], EPS_DIAG)
            out_sb = cpool.tile([2, B_LOC * 512], F32)

            # ---- per-batch inputs: vt halves first (feed main matmuls),
            # natural-V halves after (only needed by the final matmuls) ----
            vin = {}
            for b in range(B_LOC):
                vin[b] = vpool.tile([128, VIN_W], BF16, tag="vin", name=f"vin{b}")
                nc.sync.dma_start(vin[b][:, 0:2 * N], vin_d[:, b, 0:2 * N])
            for b in range(B_LOC):
                nc.sync.dma_start(vin[b][:, 2 * N:], vin_d[:, b, 2 * N:])

            # ---- PE warm-up: ~5us of dummy matmuls during the input-DMA
            # dead time.  The HAM clock gate only lifts (1.2 -> 2.4 GHz)
            # after ~3.4us of SUSTAINED matmul activity; without this the
            # whole kernel runs at half PE clock. ----
            junkw = cpool.tile([128, 128], BF16)
            nc.vector.memset(junkw[:], 1.0)
            junk = cpool.tile([128, 512], BF16)
            nc.vector.memset(junk[:], 1.0)
            for w in range(14):
                w_ps = ps_lr.tile([128, 512], F32, tag="lrps")
                nc.tensor.matmul(
                    w_ps[:], junkw[:], junk[:], start=True, stop=True,
                )

            # Per-batch state carried between the emission helpers below.
            st = {}

            def phase_a_block(b, blk):
                """Main matmuls + elementwise chain for one 512-col block."""
                s = st[b]
                f0 = blk * 512
                vt_v = vin[b][:, 0:2 * N]
                # [R; L] = W12.T @ vt  (R rows 0-63, L rows 64-127)
                lr_ps = ps_lr.tile([128, 512], F32, tag="lrps")
                for c in range(2):
                    nc.tensor.matmul(
                        lr_ps[:],
                        w12_sb[:, c * 128:(c + 1) * 128],
                        vt_v[:, c * N + f0:c * N + f0 + 512],
                        start=(c == 0), stop=(c == 1),
                    )
                # split relu (cross-base PSUM read is legal); base 0 outputs.
                nc.scalar.activation(
                    s["R"][:, f0:f0 + 512], lr_ps[0:64, :], AF.Relu,
                    bias=b12_sb[0:64, :], scale=1.0,
                )
                if blk % 2 == 0:
                    nc.scalar.activation(
                        s["L"][:, f0:f0 + 512], lr_ps[64:128, :], AF.Relu,
                        bias=b12_sb[64:128, :], scale=1.0,
                    )
                else:
                    nc.vector.tensor_scalar(
                        s["L"][:, f0:f0 + 512], lr_ps[64:128, :],
                        b12_sb[64:128, :], 0.0, OP.add, OP.max,
                    )
                # prod = L * R on DVE (block-chain critical: feeds diag)
                prod = bpool.tile([64, 512], BF16, tag="prod")
                nc.vector.tensor_tensor(
                    prod[:], s["R"][:, f0:f0 + 512],
                    s["L"][:, f0:f0 + 512], OP.mult,
                )
                # d broadcast over 64 partitions via ones-lhsT matmul
                d_ps = ps_d.tile([64, 512], F32, tag="dps")
                nc.tensor.matmul(
                    d_ps[:], ones64[:], prod[:], start=True, stop=True,
                )
                # drrep = 1/sqrt(d + eps), computed wide
                sqrep = sqpool.tile([64, 512], F32, tag="sq")
                nc.scalar.activation(
                    sqrep[:], d_ps[:], AF.Sqrt, bias=eps64[:], scale=1.0,
                )
                drrep = drpool.tile([64, 512], F32, tag="dr")
                nc.vector.reciprocal_approx_fast(drrep[:], sqrep[:])
                # ldr = L * drrep (summed for t); Rd = R * drrep so the
                # u-matmul yields c = dr*u directly.  Rd has slack -> GpSimd;
                # only the last batch's last-block ldr gates the tail -> DVE.
                if blk == NBLK - 1 and b == B_LOC - 1:
                    nc.vector.tensor_tensor(
                        s["ldr"][:, f0:f0 + 512], s["L"][:, f0:f0 + 512],
                        drrep[:], OP.mult,
                    )
                else:
                    nc.gpsimd.tensor_tensor(
                        s["ldr"][:, f0:f0 + 512], s["L"][:, f0:f0 + 512],
                        drrep[:], OP.mult,
                    )
                nc.gpsimd.tensor_tensor(
                    s["Rd"][:, f0:f0 + 512], s["R"][:, f0:f0 + 512],
                    drrep[:], OP.mult,
                )
                # partial t for this block (full t = sum of partials)
                nc.vector.tensor_reduce(
                    s["tp"][:, blk:blk + 1], s["ldr"][:, f0:f0 + 512],
                    mybir.AxisListType.X, OP.add,
                )

            def phase_b_early(b):
                """t, u-matmuls, s rows, and the DRAM scatter bounces."""
                s = st[b]
                t_f = tpool.tile([64, 1], F32, tag="tf", name=f"tf{b}")
                nc.vector.tensor_reduce(
                    t_f[:], s["tp"][:], mybir.AxisListType.X, OP.add,
                )
                t_bf = tpool.tile([64, 1], BF16, tag="tbf", name=f"tbf{b}")
                nc.scalar.activation(t_bf[:], t_f[:], AF.Copy)
                s_row = srpool.tile([1, N], BF16, tag="srow")
                s["s_col"] = scpool.tile([128, NT_B], BF16, tag="scol", name=f"scol{b}")
                for blk in range(NBLK):
                    f0 = blk * 512
                    u_ps = ps_u.tile([1, 512], F32, tag="ups")
                    nc.tensor.matmul(
                        u_ps[:], t_bf[:], s["Rd"][:, f0:f0 + 512],
                        start=True, stop=True,
                    )
                    if blk % 2 == 0:
                        nc.scalar.activation(
                            s_row[:, f0:f0 + 512], u_ps[:], AF.Copy,
                            bias=float(N + 1) / N, scale=-1.0 / N,
                        )
                    else:
                        nc.vector.tensor_scalar(
                            s_row[:, f0:f0 + 512], u_ps[:],
                            -1.0 / N, float(N + 1) / N, OP.mult, OP.add,
                        )
                # partition scatter: DRAM bounce out, then one xbar
                # transpose-DMA back ([16,128] -> [128,16] at line rate).
                s_dram = dpool.tile([1, N], BF16, tag="sdram", name=f"sdram{b}")
                nc.sync.dma_start(s_dram[:], s_row[:])
                nc.sync.dma_start_transpose(
                    s["s_col"][:], s_dram.rearrange("a (j p) -> (a j) p", p=128)
                )

            def phase_b_final(b):
                """v_mean = s^T @ V, then output.  Row-tile PAIRS share one
                [2,512] matmul (same total rows, half the instructions):
                row 0 cols 0:256 and row 1 cols 256:512 hold the two useful
                quadrants; one cross-partition PSUM add combines them."""
                s = st[b]
                vnat = vin[b][:, 2 * N:]
                v_ps = ps_v.tile([2, 512], F32, tag="vps")
                for jp in range(NT_B // 2):
                    j = 2 * jp
                    nc.tensor.matmul(
                        v_ps[:], s["s_col"][:, j:j + 2],
                        vnat[:, j * D:(j + 2) * D],
                        start=(jp == 0), stop=(jp == NT_B // 2 - 1),
                    )
                nc.scalar.activation(
                    out_sb[:, b * 512:(b + 1) * 512], v_ps[:], AF.Copy,
                )
                nc.sync.dma_start(
                    out_d[:, b * 512:(b + 1) * 512],
                    out_sb[:, b * 512:(b + 1) * 512],
                )

            def new_batch(b):
                st[b] = {
                    "R": lrpool.tile([64, N], BF16, tag="R", name=f"R{b}"),
                    "L": lrpool.tile([64, N], BF16, tag="L", name=f"L{b}"),
                    "Rd": rdpool.tile([64, N], BF16, tag="Rd", name=f"Rd{b}"),
                    "ldr": rdpool.tile([64, N], BF16, tag="ldr", name=f"ldr{b}"),
                    "tp": tpool.tile([64, NBLK], F32, tag="tp", name=f"tp{b}"),
                }

            # Software-pipelined emission at depth 2: batch b-1's u/s phase
            # and batch b-2's final matmuls queue BEHIND batch b's ready
            # main matmuls, so DMA/t-gated work never head-of-line-blocks
            # the in-order PE queue.
            for b in range(B_LOC):
                new_batch(b)
                for blk in range(NBLK):
                    phase_a_block(b, blk)
                if b >= 1:
                    phase_b_early(b - 1)
                if b >= 2:
                    phase_b_final(b - 2)
            phase_b_early(B_LOC - 1)
            phase_b_final(B_LOC - 2)
            phase_b_final(B_LOC - 1)

    nc.compile()
    return nc


def _host_prep(inputs, cfg):
    """Weight-norm, packing, casts; returns per-core input maps."""
    def wn(v, g):
        return v * (g / np.linalg.norm(v.astype(np.float64), axis=1)).astype(
            np.float32
        )[:, None]

    W1 = wn(np.asarray(inputs["U1_v"], np.float32), np.asarray(inputs["U1_g"], np.float32))
    W2 = wn(np.asarray(inputs["U2_v"], np.float32), np.asarray(inputs["U2_g"], np.float32))
    # lhsT layout [d, m]: m 0-63 -> R (W1), 64-127 -> L (W2); split d in 2 chunks
    W12T = np.concatenate([W1.T, W2.T], axis=1)          # [D, 128]
    w12 = np.ascontiguousarray(
        W12T.reshape(2, 128, 128)
    ).astype(ml_dtypes.bfloat16)                          # [c, d, m]
    w12 = np.ascontiguousarray(w12.transpose(1, 0, 2))    # [d, c, m]
    b12 = np.concatenate([
        np.asarray(inputs["U1_b"], np.float32),
        np.asarray(inputs["U2_b"], np.float32),
    ]).reshape(128, 1)

    V = np.asarray(inputs["Vmat"], np.float32)  # [B, N, D]
    Vb = V.astype(ml_dtypes.bfloat16)
    in_maps = []
    for k in range(NCORES):
        packs = []
        for b in range(B_LOC):
            Vk = Vb[k * B_LOC + b]                                    # [N, D]
            vt = Vk.T.reshape(2, 128, N).transpose(1, 0, 2).reshape(128, 2 * N)
            vn = Vk.reshape(NT_B, 128, D).transpose(1, 0, 2).reshape(128, NT_B * D)
            packs.append(np.concatenate([vt, vn], axis=1))            # [128, VIN_W]
        vin = np.ascontiguousarray(np.stack(packs, axis=1))           # [128, B_LOC, VIN_W]
        in_maps.append({"vin": vin, "w12": w12, "b12": b12})
    return in_maps


def _epilogue(v_mean, inputs):
    """feat = v_mean @ W_lin.T + b_lin, then training-mode batchnorm."""
    W_lin = np.asarray(inputs["W_lin"], np.float32)
    b_lin = np.asarray(inputs["b_lin"], np.float32)
    gamma = np.asarray(inputs["gamma"], np.float32)
    beta = np.asarray(inputs["beta"], np.float32)
    feat = v_mean.astype(np.float32) @ W_lin.T + b_lin
    mu = feat.mean(axis=0)
    var = feat.var(axis=0)
    out = (feat - mu) / np.sqrt(var + EPS_BN) * gamma + beta
    return out.astype(np.float32)


def kernel(**inputs):
    cfg = dict(CONFIG)
    key = ("v2",)
    if key not in _CACHE:
        _CACHE[key] = _build(cfg)
    nc = _CACHE[key]
    in_maps = _host_prep(inputs, cfg)
    res = run_bass_kernel_spmd(
        nc, in_maps, core_ids=list(range(NCORES)), trace=cfg["trace"]
    )
    kernel.last_results = res
    v_parts = []
    for k in range(NCORES):
        x = res.results[k]["vmean"].reshape(2, B_LOC, 512)
        v_parts.append(x[0, :, 0:256] + x[1, :, 256:512])
    v_mean = np.concatenate(v_parts, axis=0)
    return _epilogue(v_mean, inputs)


# revision 25
# speedup vs baseline: 1.0240x; 1.0240x over previous
"""Trainium2 Bass kernel: Encoder_HieStackedCorr (UnCorrVmat_Detail, t_method='uncorr').

Math (per batch b):
    W1 = wn(U1_v, U1_g); W2 = wn(U2_v, U2_g)
    R = relu(V @ W1.T + b1)          [N, LR]
    L = relu(V @ W2.T + b2)          [N, LR]
    UnCorr = L @ R.T                 [N, N]
    d[n] = UnCorr[n, n] = sum_l L[n,l] R[n,l]
    dr = 1/sqrt(d + eps)
    A = 1 + I - dr dr^T * UnCorr
    v = mean_n (A @ V) = (1/N) * s @ V  where s[m] = N + 1 - dr[m] * (t . R[m,:]),
                                              t = sum_n dr[n] L[n,:]
    feat = v @ W_lin.T + b_lin ; out = batchnorm(feat)   (training-mode stats)

The N x N matrix is never materialized: mean-pooling commutes with the matmul,
collapsing the O(B N^2 (LR+D)) reference into O(B N D LR) work.

Kernel structure (v2):
  - W1 || W2 stacked into one [128, 128] lhsT: a single matmul pair per
    512-block produces both R (partitions 0-63) and L (64-127); one fused
    relu+bias on the whole [128, 512] PSUM tile.
  - Diag via ones[64,64] lhsT: d arrives already broadcast over 64
    partitions ([64, 512] PSUM), so rsqrt runs wide (ACT sqrt + DVE
    reciprocal_approx_fast) and no separate broadcast matmul is needed.
  - t via chained tensor_tensor_reduce (mult+add accumulate across blocks).
  - dr is folded into R (Rd = R * drrep on GpSimd), so the u-matmul yields
    c = dr*u directly and s = (N+1-c)/N comes from one ACT affine per block.
  - s scattered to column layout via a DRAM bounce, then 16 accumulating
    matmuls against natural-layout V give v_mean.

Sharding: data-parallel over batch, 4 batches per core on 8 cores.  The tiny
[32,256] linear + batchnorm epilogue (cross-core batch stats) runs on host.
"""

import os
import sys

import numpy as np

for _p in ("/opt/trn_rl_repo", "/root/.axon_site/_ro/trn_rl_repo"):
    if os.path.isdir(_p) and _p not in sys.path:
        sys.path.insert(0, _p)
        break

import ml_dtypes  # noqa: E402
import concourse.bass as bass  # noqa: E402
import concourse.bacc as bacc  # noqa: E402
import concourse.mybir as mybir  # noqa: E402
import concourse.tile as tile  # noqa: E402
from concourse.bass_utils import run_bass_kernel_spmd  # noqa: E402


def _ensure_ntff_hook():
    """Shim the missing ``antenv.axon_hooks`` registry so trace=True works."""
    import types

    try:
        from antenv.axon_hooks import get_axon_ntff_profile_hook  # noqa: F401
        return
    except ImportError:
        pass
    try:
        from trn_agent_boot.trn_boot import _ntff_profile_via_ctypes
        hook = _ntff_profile_via_ctypes("/opt/axon/libaxon_pjrt.so")
    except Exception:
        hook = None
    mod = types.ModuleType("antenv.axon_hooks")
    mod._hook = hook
    mod.get_axon_ntff_profile_hook = lambda: mod._hook
    mod.set_axon_ntff_profile_hook = lambda h: setattr(mod, "_hook", h)
    sys.modules["antenv.axon_hooks"] = mod


_ensure_ntff_hook()

# Problem constants (hardcoded; see module docstring).
B, N, D, LR, EMB = 32, 2048, 256, 64, 256
NCORES = 8
B_LOC = B // NCORES          # 4 batches per core
NT_B = N // 128              # 16 row-tiles per batch
NBLK = N // 512              # 4 512-col blocks per batch
VIN_W = 2 * N + NT_B * D     # per-batch packed width: vt (2*N) + v-nat (16*D)
EPS_DIAG = 1e-6
EPS_BN = 1e-5

F32 = mybir.dt.float32
BF16 = mybir.dt.bfloat16

CONFIG = dict(dt="bf16", trace=False)

_CACHE = {}


def _build(cfg):
    nc = bacc.Bacc("TRN2", target_bir_lowering=False, debug=False)

    vin_d = nc.dram_tensor("vin", [128, B_LOC, VIN_W], BF16, kind="ExternalInput").ap()
    w12_d = nc.dram_tensor("w12", [128, 2, 128], BF16, kind="ExternalInput").ap()
    b12_d = nc.dram_tensor("b12", [128, 1], F32, kind="ExternalInput").ap()
    out_d = nc.dram_tensor("vmean", [2, B_LOC * 512], F32, kind="ExternalOutput").ap()

    AF = mybir.ActivationFunctionType
    OP = mybir.AluOpType

    with tile.TileContext(nc) as tc:
        with (
            tc.tile_pool(name="const", bufs=1) as cpool,
            tc.tile_pool(name="vin", bufs=B_LOC) as vpool,
            tc.tile_pool(name="lr", bufs=3) as lrpool,
            tc.tile_pool(name="rd", bufs=3) as rdpool,
            tc.tile_pool(name="blk", bufs=3) as bpool,
            tc.tile_pool(name="sq", bufs=3) as sqpool,
            tc.tile_pool(name="dr", bufs=3) as drpool,
            tc.tile_pool(name="tsm", bufs=12) as tpool,
            tc.tile_pool(name="srow", bufs=3) as srpool,
            tc.tile_pool(name="scol", bufs=3) as scpool,
            tc.tile_pool(name="ps_lr", bufs=4, space="PSUM") as ps_lr,
            tc.tile_pool(name="ps_d", bufs=2, space="PSUM") as ps_d,
            tc.tile_pool(name="ps_u", bufs=1, space="PSUM") as ps_u,
            tc.tile_pool(name="ps_v", bufs=1, space="PSUM") as ps_v,
            tc.tile_pool(name="dram", bufs=4, space="DRAM") as dpool,
        ):
            # ---- constants / weights ----
            w12_sb = cpool.tile([128, 2 * 128], BF16)
            nc.sync.dma_start(
                w12_sb[:].rearrange("p (c m) -> p c m", c=2), w12_d[:]
            )
            b12_sb = cpool.tile([128, 1], F32)
            nc.sync.dma_start(b12_sb[:], b12_d[:])
            ones64 = cpool.tile([64, 64], BF16)
            nc.vector.memset(ones64[:], 1.0)
            eps64 = cpool.tile([64, 1], F32)
            nc.vector.memset(eps64[:], EPS_DIAG)
            out_sb = cpool.tile([2, B_LOC * 512], F32)

            # ---- per-batch inputs: vt halves first (feed main matmuls),
            # natural-V halves after (only needed by the final matmuls) ----
            vin = {}
            for b in range(B_LOC):
                vin[b] = vpool.tile([128, VIN_W], BF16, tag="vin", name=f"vin{b}")
                nc.sync.dma_start(vin[b][:, 0:2 * N], vin_d[:, b, 0:2 * N])
            for b in range(B_LOC):
                nc.sync.dma_start(vin[b][:, 2 * N:], vin_d[:, b, 2 * N:])

            # ---- PE warm-up: ~5us of dummy matmuls during the input-DMA
            # dead time.  The HAM clock gate only lifts (1.2 -> 2.4 GHz)
            # after ~3.4us of SUSTAINED matmul activity; without this the
            # whole kernel runs at half PE clock. ----
            junkw = cpool.tile([128, 128], BF16)
            nc.vector.memset(junkw[:], 1.0)
            junk = cpool.tile([128, 512], BF16)
            nc.vector.memset(junk[:], 1.0)
            for w in range(14):
                w_ps = ps_lr.tile([128, 512], F32, tag="lrps")
                nc.tensor.matmul(
                    w_ps[:], junkw[:], junk[:], start=True, stop=True,
                )

            # Per-batch state carried between the emission helpers below.
            st = {}

            def phase_a_mains(b):
                """All 8 main matmuls of a batch, back-to-back: a ~3.4us
                contiguous PE burst that (re)lifts the HAM clock gate."""
                s = st[b]
                vt_v = vin[b][:, 0:2 * N]
                s["lr_ps"] = []
                for blk in range(NBLK):
                    f0 = blk * 512
                    lr_ps = ps_lr.tile([128, 512], F32, tag="lrps")
                    s["lr_ps"].append(lr_ps)
                    for c in range(2):
                        nc.tensor.matmul(
                            lr_ps[:],
                            w12_sb[:, c * 128:(c + 1) * 128],
                            vt_v[:, c * N + f0:c * N + f0 + 512],
                            start=(c == 0), stop=(c == 1),
                        )

            def phase_a_block(b, blk):
                """Elementwise chain + small matmuls for one 512-col block."""
                s = st[b]
                f0 = blk * 512
                lr_ps = s["lr_ps"][blk]
                # split relu (cross-base PSUM read is legal); base 0 outputs.
                nc.scalar.activation(
                    s["R"][:, f0:f0 + 512], lr_ps[0:64, :], AF.Relu,
                    bias=b12_sb[0:64, :], scale=1.0,
                )
                if blk % 2 == 0:
                    nc.scalar.activation(
                        s["L"][:, f0:f0 + 512], lr_ps[64:128, :], AF.Relu,
                        bias=b12_sb[64:128, :], scale=1.0,
                    )
                else:
                    nc.vector.tensor_scalar(
                        s["L"][:, f0:f0 + 512], lr_ps[64:128, :],
                        b12_sb[64:128, :], 0.0, OP.add, OP.max,
                    )
                # prod = L * R on DVE (block-chain critical: feeds diag)
                prod = bpool.tile([64, 512], BF16, tag="prod")
                nc.vector.tensor_tensor(
                    prod[:], s["R"][:, f0:f0 + 512],
                    s["L"][:, f0:f0 + 512], OP.mult,
                )
                # d broadcast over 64 partitions via ones-lhsT matmul
                d_ps = ps_d.tile([64, 512], F32, tag="dps")
                nc.tensor.matmul(
                    d_ps[:], ones64[:], prod[:], start=True, stop=True,
                )
                # drrep = 1/sqrt(d + eps), computed wide
                sqrep = sqpool.tile([64, 512], F32, tag="sq")
                nc.scalar.activation(
                    sqrep[:], d_ps[:], AF.Sqrt, bias=eps64[:], scale=1.0,
                )
                drrep = drpool.tile([64, 512], F32, tag="dr")
                nc.vector.reciprocal_approx_fast(drrep[:], sqrep[:])
                # ldr = L * drrep (summed for t); Rd = R * drrep so the
                # u-matmul yields c = dr*u directly.  Rd has slack -> GpSimd;
                # only the last batch's last-block ldr gates the tail -> DVE.
                if blk == NBLK - 1 and b == B_LOC - 1:
                    nc.vector.tensor_tensor(
                        s["ldr"][:, f0:f0 + 512], s["L"][:, f0:f0 + 512],
                        drrep[:], OP.mult,
                    )
                else:
                    nc.gpsimd.tensor_tensor(
                        s["ldr"][:, f0:f0 + 512], s["L"][:, f0:f0 + 512],
                        drrep[:], OP.mult,
                    )
                nc.gpsimd.tensor_tensor(
                    s["Rd"][:, f0:f0 + 512], s["R"][:, f0:f0 + 512],
                    drrep[:], OP.mult,
                )
                # partial t for this block (full t = sum of partials)
                nc.vector.tensor_reduce(
                    s["tp"][:, blk:blk + 1], s["ldr"][:, f0:f0 + 512],
                    mybir.AxisListType.X, OP.add,
                )

            def phase_b_early(b):
                """t, u-matmuls, s rows, and the DRAM scatter bounces."""
                s = st[b]
                t_f = tpool.tile([64, 1], F32, tag="tf", name=f"tf{b}")
                nc.vector.tensor_reduce(
                    t_f[:], s["tp"][:], mybir.AxisListType.X, OP.add,
                )
                t_bf = tpool.tile([64, 1], BF16, tag="tbf", name=f"tbf{b}")
                nc.scalar.activation(t_bf[:], t_f[:], AF.Copy)
                s_row = srpool.tile([1, N], BF16, tag="srow")
                s["s_col"] = scpool.tile([128, NT_B], BF16, tag="scol", name=f"scol{b}")
                for blk in range(NBLK):
                    f0 = blk * 512
                    u_ps = ps_u.tile([1, 512], F32, tag="ups")
                    nc.tensor.matmul(
                        u_ps[:], t_bf[:], s["Rd"][:, f0:f0 + 512],
                        start=True, stop=True,
                    )
                    if blk % 2 == 0:
                        nc.scalar.activation(
                            s_row[:, f0:f0 + 512], u_ps[:], AF.Copy,
                            bias=float(N + 1) / N, scale=-1.0 / N,
                        )
                    else:
                        nc.vector.tensor_scalar(
                            s_row[:, f0:f0 + 512], u_ps[:],
                            -1.0 / N, float(N + 1) / N, OP.mult, OP.add,
                        )
                # partition scatter: DRAM bounce out, then one xbar
                # transpose-DMA back ([16,128] -> [128,16] at line rate).
                s_dram = dpool.tile([1, N], BF16, tag="sdram", name=f"sdram{b}")
                nc.sync.dma_start(s_dram[:], s_row[:])
                nc.sync.dma_start_transpose(
                    s["s_col"][:], s_dram.rearrange("a (j p) -> (a j) p", p=128)
                )

            def phase_b_final(b):
                """v_mean = s^T @ V, then output.  Row-tile PAIRS share one
                [2,512] matmul (same total rows, half the instructions):
                row 0 cols 0:256 and row 1 cols 256:512 hold the two useful
                quadrants; one cross-partition PSUM add combines them."""
                s = st[b]
                vnat = vin[b][:, 2 * N:]
                v_ps = ps_v.tile([2, 512], F32, tag="vps")
                for jp in range(NT_B // 2):
                    j = 2 * jp
                    nc.tensor.matmul(
                        v_ps[:], s["s_col"][:, j:j + 2],
                        vnat[:, j * D:(j + 2) * D],
                        start=(jp == 0), stop=(jp == NT_B // 2 - 1),
                    )
                nc.scalar.activation(
                    out_sb[:, b * 512:(b + 1) * 512], v_ps[:], AF.Copy,
                )
                nc.sync.dma_start(
                    out_d[:, b * 512:(b + 1) * 512],
                    out_sb[:, b * 512:(b + 1) * 512],
                )

            def new_batch(b):
                st[b] = {
                    "R": lrpool.tile([64, N], BF16, tag="R", name=f"R{b}"),
                    "L": lrpool.tile([64, N], BF16, tag="L", name=f"L{b}"),
                    "Rd": rdpool.tile([64, N], BF16, tag="Rd", name=f"Rd{b}"),
                    "ldr": rdpool.tile([64, N], BF16, tag="ldr", name=f"ldr{b}"),
                    "tp": tpool.tile([64, NBLK], F32, tag="tp", name=f"tp{b}"),
                }

            # Software-pipelined emission at depth 2: batch b-1's u/s phase
            # and batch b-2's final matmuls queue BEHIND batch b's ready
            # main matmuls, so DMA/t-gated work never head-of-line-blocks
            # the in-order PE queue.
            for b in range(B_LOC):
                new_batch(b)
                phase_a_mains(b)
                for blk in range(NBLK):
                    phase_a_block(b, blk)
                if b >= 1:
                    phase_b_early(b - 1)
                if b >= 2:
                    phase_b_final(b - 2)
            phase_b_early(B_LOC - 1)
            phase_b_final(B_LOC - 2)
            phase_b_final(B_LOC - 1)

    nc.compile()
    return nc


def _host_prep(inputs, cfg):
    """Weight-norm, packing, casts; returns per-core input maps."""
    def wn(v, g):
        return v * (g / np.linalg.norm(v.astype(np.float64), axis=1)).astype(
            np.float32
        )[:, None]

    W1 = wn(np.asarray(inputs["U1_v"], np.float32), np.asarray(inputs["U1_g"], np.float32))
    W2 = wn(np.asarray(inputs["U2_v"], np.float32), np.asarray(inputs["U2_g"], np.float32))
    # lhsT layout [d, m]: m 0-63 -> R (W1), 64-127 -> L (W2); split d in 2 chunks
    W12T = np.concatenate([W1.T, W2.T], axis=1)          # [D, 128]
    w12 = np.ascontiguousarray(
        W12T.reshape(2, 128, 128)
    ).astype(ml_dtypes.bfloat16)                          # [c, d, m]
    w12 = np.ascontiguousarray(w12.transpose(1, 0, 2))    # [d, c, m]
    b12 = np.concatenate([
        np.asarray(inputs["U1_b"], np.float32),
        np.asarray(inputs["U2_b"], np.float32),
    ]).reshape(128, 1)

    V = np.asarray(inputs["Vmat"], np.float32)  # [B, N, D]
    Vb = V.astype(ml_dtypes.bfloat16)
    in_maps = []
    for k in range(NCORES):
        packs = []
        for b in range(B_LOC):
            Vk = Vb[k * B_LOC + b]                                    # [N, D]
            vt = Vk.T.reshape(2, 128, N).transpose(1, 0, 2).reshape(128, 2 * N)
            vn = Vk.reshape(NT_B, 128, D).transpose(1, 0, 2).reshape(128, NT_B * D)
            packs.append(np.concatenate([vt, vn], axis=1))            # [128, VIN_W]
        vin = np.ascontiguousarray(np.stack(packs, axis=1))           # [128, B_LOC, VIN_W]
        in_maps.append({"vin": vin, "w12": w12, "b12": b12})
    return in_maps


def _epilogue(v_mean, inputs):
    """feat = v_mean @ W_lin.T + b_lin, then training-mode batchnorm."""
    W_lin = np.asarray(inputs["W_lin"], np.float32)
    b_lin = np.asarray(inputs["b_lin"], np.float32)
    gamma = np.asarray(inputs["gamma"], np.float32)
    beta = np.asarray(inputs["beta"], np.float32)
    feat = v_mean.astype(np.float32) @ W_lin.T + b_lin
    mu = feat.mean(axis=0)
    var = feat.var(axis=0)
    out = (feat - mu) / np.sqrt(var + EPS_BN) * gamma + beta
    return out.astype(np.float32)


def kernel(**inputs):
    cfg = dict(CONFIG)
    key = ("v2",)
    if key not in _CACHE:
        _CACHE[key] = _build(cfg)
    nc = _CACHE[key]
    in_maps = _host_prep(inputs, cfg)
    res = run_bass_kernel_spmd(
        nc, in_maps, core_ids=list(range(NCORES)), trace=cfg["trace"]
    )
    kernel.last_results = res
    v_parts = []
    for k in range(NCORES):
        x = res.results[k]["vmean"].reshape(2, B_LOC, 512)
        v_parts.append(x[0, :, 0:256] + x[1, :, 256:512])
    v_mean = np.concatenate(v_parts, axis=0)
    return _epilogue(v_mean, inputs)


# revision 26
# speedup vs baseline: 1.1353x; 1.1087x over previous
"""Trainium2 Bass kernel: Encoder_HieStackedCorr (UnCorrVmat_Detail, t_method='uncorr').

Math (per batch b):
    W1 = wn(U1_v, U1_g); W2 = wn(U2_v, U2_g)
    R = relu(V @ W1.T + b1)          [N, LR]
    L = relu(V @ W2.T + b2)          [N, LR]
    UnCorr = L @ R.T                 [N, N]
    d[n] = UnCorr[n, n] = sum_l L[n,l] R[n,l]
    dr = 1/sqrt(d + eps)
    A = 1 + I - dr dr^T * UnCorr
    v = mean_n (A @ V) = (1/N) * s @ V  where s[m] = N + 1 - dr[m] * (t . R[m,:]),
                                              t = sum_n dr[n] L[n,:]
    feat = v @ W_lin.T + b_lin ; out = batchnorm(feat)   (training-mode stats)

The N x N matrix is never materialized: mean-pooling commutes with the matmul,
collapsing the O(B N^2 (LR+D)) reference into O(B N D LR) work.

Kernel structure (v2):
  - W1 || W2 stacked into one [128, 128] lhsT: a single matmul pair per
    512-block produces both R (partitions 0-63) and L (64-127); one fused
    relu+bias on the whole [128, 512] PSUM tile.
  - Diag via ones[64,64] lhsT: d arrives already broadcast over 64
    partitions ([64, 512] PSUM), so rsqrt runs wide (ACT sqrt + DVE
    reciprocal_approx_fast) and no separate broadcast matmul is needed.
  - t via chained tensor_tensor_reduce (mult+add accumulate across blocks).
  - dr is folded into R (Rd = R * drrep on GpSimd), so the u-matmul yields
    c = dr*u directly and s = (N+1-c)/N comes from one ACT affine per block.
  - s scattered to column layout via a DRAM bounce, then 16 accumulating
    matmuls against natural-layout V give v_mean.

Sharding: data-parallel over batch, 4 batches per core on 8 cores.  The tiny
[32,256] linear + batchnorm epilogue (cross-core batch stats) runs on host.
"""

import os
import sys

import numpy as np

for _p in ("/opt/trn_rl_repo", "/root/.axon_site/_ro/trn_rl_repo"):
    if os.path.isdir(_p) and _p not in sys.path:
        sys.path.insert(0, _p)
        break

import ml_dtypes  # noqa: E402
import concourse.bass as bass  # noqa: E402
import concourse.bacc as bacc  # noqa: E402
import concourse.mybir as mybir  # noqa: E402
import concourse.tile as tile  # noqa: E402
from concourse.bass_utils import run_bass_kernel_spmd  # noqa: E402


def _ensure_ntff_hook():
    """Shim the missing ``antenv.axon_hooks`` registry so trace=True works."""
    import types

    try:
        from antenv.axon_hooks import get_axon_ntff_profile_hook  # noqa: F401
        return
    except ImportError:
        pass
    try:
        from trn_agent_boot.trn_boot import _ntff_profile_via_ctypes
        hook = _ntff_profile_via_ctypes("/opt/axon/libaxon_pjrt.so")
    except Exception:
        hook = None
    mod = types.ModuleType("antenv.axon_hooks")
    mod._hook = hook
    mod.get_axon_ntff_profile_hook = lambda: mod._hook
    mod.set_axon_ntff_profile_hook = lambda h: setattr(mod, "_hook", h)
    sys.modules["antenv.axon_hooks"] = mod


_ensure_ntff_hook()

# Problem constants (hardcoded; see module docstring).
B, N, D, LR, EMB = 32, 2048, 256, 64, 256
NCORES = 8
B_LOC = B // NCORES          # 4 batches per core
NT_B = N // 128              # 16 row-tiles per batch
NBLK = N // 512              # 4 512-col blocks per batch
VIN_W = 2 * N + NT_B * D     # per-batch packed width: vt (2*N) + v-nat (16*D)
EPS_DIAG = 1e-6
EPS_BN = 1e-5

F32 = mybir.dt.float32
BF16 = mybir.dt.bfloat16

CONFIG = dict(dt="bf16", trace=False)

_CACHE = {}


def _build(cfg):
    nc = bacc.Bacc("TRN2", target_bir_lowering=False, debug=False)

    vin_d = nc.dram_tensor("vin", [128, B_LOC, VIN_W], BF16, kind="ExternalInput").ap()
    w12_d = nc.dram_tensor("w12", [128, 2, 128], BF16, kind="ExternalInput").ap()
    b12_d = nc.dram_tensor("b12", [128, 1], F32, kind="ExternalInput").ap()
    out_d = nc.dram_tensor("vmean", [2, B_LOC * 512], F32, kind="ExternalOutput").ap()

    AF = mybir.ActivationFunctionType
    OP = mybir.AluOpType

    with tile.TileContext(nc) as tc:
        with (
            tc.tile_pool(name="const", bufs=1) as cpool,
            tc.tile_pool(name="vin", bufs=B_LOC) as vpool,
            tc.tile_pool(name="lr", bufs=3) as lrpool,
            tc.tile_pool(name="rd", bufs=3) as rdpool,
            tc.tile_pool(name="blk", bufs=3) as bpool,
            tc.tile_pool(name="sq", bufs=3) as sqpool,
            tc.tile_pool(name="dr", bufs=3) as drpool,
            tc.tile_pool(name="tsm", bufs=12) as tpool,
            tc.tile_pool(name="srow", bufs=3) as srpool,
            tc.tile_pool(name="scol", bufs=3) as scpool,
            tc.tile_pool(name="ps_lr", bufs=4, space="PSUM") as ps_lr,
            tc.tile_pool(name="ps_d", bufs=2, space="PSUM") as ps_d,
            tc.tile_pool(name="ps_u", bufs=1, space="PSUM") as ps_u,
            tc.tile_pool(name="ps_v", bufs=1, space="PSUM") as ps_v,
            tc.tile_pool(name="dram", bufs=4, space="DRAM") as dpool,
        ):
            # ---- constants / weights ----
            w12_sb = cpool.tile([128, 2 * 128], BF16)
            nc.sync.dma_start(
                w12_sb[:].rearrange("p (c m) -> p c m", c=2), w12_d[:]
            )
            b12_sb = cpool.tile([128, 1], F32)
            nc.sync.dma_start(b12_sb[:], b12_d[:])
            ones64 = cpool.tile([64, 64], BF16)
            nc.vector.memset(ones64[:], 1.0)
            eps64 = cpool.tile([64, 1], F32)
            nc.vector.memset(eps64[:], EPS_DIAG)
            out_sb = cpool.tile([2, B_LOC * 512], F32)

            # ---- per-batch inputs: vt halves first (feed main matmuls),
            # natural-V halves after (only needed by the final matmuls) ----
            vin = {}
            for b in range(B_LOC):
                vin[b] = vpool.tile([128, VIN_W], BF16, tag="vin", name=f"vin{b}")
                nc.sync.dma_start(vin[b][:, 0:2 * N], vin_d[:, b, 0:2 * N])
            for b in range(B_LOC):
                nc.sync.dma_start(vin[b][:, 2 * N:], vin_d[:, b, 2 * N:])

            # ---- PE warm-up: ~5us of dummy matmuls during the input-DMA
            # dead time.  The HAM clock gate only lifts (1.2 -> 2.4 GHz)
            # after ~3.4us of SUSTAINED matmul activity; without this the
            # whole kernel runs at half PE clock. ----
            junkw = cpool.tile([128, 128], BF16)
            nc.vector.memset(junkw[:], 1.0)
            junk = cpool.tile([128, 512], BF16)
            nc.vector.memset(junk[:], 1.0)
            for w in range(14):
                w_ps = ps_lr.tile([128, 512], F32, tag="lrps")
                nc.tensor.matmul(
                    w_ps[:], junkw[:], junk[:], start=True, stop=True,
                )

            # Per-batch state carried between the emission helpers below.
            st = {}

            def phase_a_mains(b):
                """All 8 main matmuls of a batch, back-to-back: a ~3.4us
                contiguous PE burst that (re)lifts the HAM clock gate."""
                s = st[b]
                vt_v = vin[b][:, 0:2 * N]
                s["lr_ps"] = []
                for blk in range(NBLK):
                    f0 = blk * 512
                    lr_ps = ps_lr.tile([128, 512], F32, tag="lrps")
                    s["lr_ps"].append(lr_ps)
                    for c in range(2):
                        nc.tensor.matmul(
                            lr_ps[:],
                            w12_sb[:, c * 128:(c + 1) * 128],
                            vt_v[:, c * N + f0:c * N + f0 + 512],
                            start=(c == 0), stop=(c == 1),
                        )

            def phase_a_block(b, blk):
                """Elementwise chain + small matmuls for one 512-col block."""
                s = st[b]
                f0 = blk * 512
                lr_ps = s["lr_ps"][blk]
                # split relu (cross-base PSUM read is legal); base 0 outputs.
                nc.scalar.activation(
                    s["R"][:, f0:f0 + 512], lr_ps[0:64, :], AF.Relu,
                    bias=b12_sb[0:64, :], scale=1.0,
                )
                if blk % 2 == 0:
                    nc.scalar.activation(
                        s["L"][:, f0:f0 + 512], lr_ps[64:128, :], AF.Relu,
                        bias=b12_sb[64:128, :], scale=1.0,
                    )
                else:
                    nc.vector.tensor_scalar(
                        s["L"][:, f0:f0 + 512], lr_ps[64:128, :],
                        b12_sb[64:128, :], 0.0, OP.add, OP.max,
                    )
                # prod = L * R on DVE (block-chain critical: feeds diag)
                prod = bpool.tile([64, 512], BF16, tag="prod")
                nc.vector.tensor_tensor(
                    prod[:], s["R"][:, f0:f0 + 512],
                    s["L"][:, f0:f0 + 512], OP.mult,
                )
                # d broadcast over 64 partitions via ones-lhsT matmul
                d_ps = ps_d.tile([64, 512], F32, tag="dps")
                nc.tensor.matmul(
                    d_ps[:], ones64[:], prod[:], start=True, stop=True,
                )
                # drrep = 1/sqrt(d + eps), computed wide
                sqrep = sqpool.tile([64, 512], F32, tag="sq")
                nc.scalar.activation(
                    sqrep[:], d_ps[:], AF.Sqrt, bias=eps64[:], scale=1.0,
                )
                nc.vector.reciprocal_approx_fast(
                    s["drrep"][:, f0:f0 + 512], sqrep[:]
                )

            def phase_a_post(b):
                """Batch-wide dr products on GpSimd (one big op amortizes the
                ~0.5us Q7 per-op overhead) ; ldr feeds the t reduction."""
                s = st[b]
                nc.gpsimd.tensor_tensor(
                    s["ldr"][:], s["L"][:], s["drrep"][:], OP.mult,
                )
                nc.gpsimd.tensor_tensor(
                    s["Rd"][:], s["R"][:], s["drrep"][:], OP.mult,
                )

            def phase_b_early(b):
                """t, u-matmuls, s rows, and the DRAM scatter bounces."""
                s = st[b]
                t_f = tpool.tile([64, 1], F32, tag="tf", name=f"tf{b}")
                nc.vector.tensor_reduce(
                    t_f[:], s["ldr"][:], mybir.AxisListType.X, OP.add,
                )
                t_bf = tpool.tile([64, 1], BF16, tag="tbf", name=f"tbf{b}")
                nc.scalar.activation(t_bf[:], t_f[:], AF.Copy)
                s_row = srpool.tile([1, N], BF16, tag="srow")
                s["s_col"] = scpool.tile([128, NT_B], BF16, tag="scol", name=f"scol{b}")
                for blk in range(NBLK):
                    f0 = blk * 512
                    u_ps = ps_u.tile([1, 512], F32, tag="ups")
                    nc.tensor.matmul(
                        u_ps[:], t_bf[:], s["Rd"][:, f0:f0 + 512],
                        start=True, stop=True,
                    )
                    if blk % 2 == 0:
                        nc.scalar.activation(
                            s_row[:, f0:f0 + 512], u_ps[:], AF.Copy,
                            bias=float(N + 1) / N, scale=-1.0 / N,
                        )
                    else:
                        nc.vector.tensor_scalar(
                            s_row[:, f0:f0 + 512], u_ps[:],
                            -1.0 / N, float(N + 1) / N, OP.mult, OP.add,
                        )
                # partition scatter: DRAM bounce out, then one xbar
                # transpose-DMA back ([16,128] -> [128,16] at line rate).
                s_dram = dpool.tile([1, N], BF16, tag="sdram", name=f"sdram{b}")
                nc.sync.dma_start(s_dram[:], s_row[:])
                nc.sync.dma_start_transpose(
                    s["s_col"][:], s_dram.rearrange("a (j p) -> (a j) p", p=128)
                )

            def phase_b_final(b):
                """v_mean = s^T @ V, then output.  Row-tile PAIRS share one
                [2,512] matmul (same total rows, half the instructions):
                row 0 cols 0:256 and row 1 cols 256:512 hold the two useful
                quadrants; one cross-partition PSUM add combines them."""
                s = st[b]
                vnat = vin[b][:, 2 * N:]
                v_ps = ps_v.tile([2, 512], F32, tag="vps")
                for jp in range(NT_B // 2):
                    j = 2 * jp
                    nc.tensor.matmul(
                        v_ps[:], s["s_col"][:, j:j + 2],
                        vnat[:, j * D:(j + 2) * D],
                        start=(jp == 0), stop=(jp == NT_B // 2 - 1),
                    )
                nc.scalar.activation(
                    out_sb[:, b * 512:(b + 1) * 512], v_ps[:], AF.Copy,
                )
                nc.sync.dma_start(
                    out_d[:, b * 512:(b + 1) * 512],
                    out_sb[:, b * 512:(b + 1) * 512],
                )

            def new_batch(b):
                st[b] = {
                    "R": lrpool.tile([64, N], BF16, tag="R", name=f"R{b}"),
                    "L": lrpool.tile([64, N], BF16, tag="L", name=f"L{b}"),
                    "Rd": rdpool.tile([64, N], BF16, tag="Rd", name=f"Rd{b}"),
                    "ldr": rdpool.tile([64, N], BF16, tag="ldr", name=f"ldr{b}"),
                    "drrep": drpool.tile([64, N], F32, tag="dr", name=f"dr{b}"),
                }

            # Software-pipelined emission at depth 2: batch b-1's u/s phase
            # and batch b-2's final matmuls queue BEHIND batch b's ready
            # main matmuls, so DMA/t-gated work never head-of-line-blocks
            # the in-order PE queue.
            for b in range(B_LOC):
                new_batch(b)
                phase_a_mains(b)
                phase_a_block(b, 0)
                phase_a_block(b, 1)
                if b >= 2:
                    phase_b_final(b - 2)
                phase_a_block(b, 2)
                phase_a_block(b, 3)
                phase_a_post(b)
                if b >= 1:
                    phase_b_early(b - 1)
            phase_b_early(B_LOC - 1)
            phase_b_final(B_LOC - 2)
            phase_b_final(B_LOC - 1)

    nc.compile()
    return nc


def _host_prep(inputs, cfg):
    """Weight-norm, packing, casts; returns per-core input maps."""
    def wn(v, g):
        return v * (g / np.linalg.norm(v.astype(np.float64), axis=1)).astype(
            np.float32
        )[:, None]

    W1 = wn(np.asarray(inputs["U1_v"], np.float32), np.asarray(inputs["U1_g"], np.float32))
    W2 = wn(np.asarray(inputs["U2_v"], np.float32), np.asarray(inputs["U2_g"], np.float32))
    # lhsT layout [d, m]: m 0-63 -> R (W1), 64-127 -> L (W2); split d in 2 chunks
    W12T = np.concatenate([W1.T, W2.T], axis=1)          # [D, 128]
    w12 = np.ascontiguousarray(
        W12T.reshape(2, 128, 128)
    ).astype(ml_dtypes.bfloat16)                          # [c, d, m]
    w12 = np.ascontiguousarray(w12.transpose(1, 0, 2))    # [d, c, m]
    b12 = np.concatenate([
        np.asarray(inputs["U1_b"], np.float32),
        np.asarray(inputs["U2_b"], np.float32),
    ]).reshape(128, 1)

    V = np.asarray(inputs["Vmat"], np.float32)  # [B, N, D]
    Vb = V.astype(ml_dtypes.bfloat16)
    in_maps = []
    for k in range(NCORES):
        packs = []
        for b in range(B_LOC):
            Vk = Vb[k * B_LOC + b]                                    # [N, D]
            vt = Vk.T.reshape(2, 128, N).transpose(1, 0, 2).reshape(128, 2 * N)
            vn = Vk.reshape(NT_B, 128, D).transpose(1, 0, 2).reshape(128, NT_B * D)
            packs.append(np.concatenate([vt, vn], axis=1))            # [128, VIN_W]
        vin = np.ascontiguousarray(np.stack(packs, axis=1))           # [128, B_LOC, VIN_W]
        in_maps.append({"vin": vin, "w12": w12, "b12": b12})
    return in_maps


def _epilogue(v_mean, inputs):
    """feat = v_mean @ W_lin.T + b_lin, then training-mode batchnorm."""
    W_lin = np.asarray(inputs["W_lin"], np.float32)
    b_lin = np.asarray(inputs["b_lin"], np.float32)
    gamma = np.asarray(inputs["gamma"], np.float32)
    beta = np.asarray(inputs["beta"], np.float32)
    feat = v_mean.astype(np.float32) @ W_lin.T + b_lin
    mu = feat.mean(axis=0)
    var = feat.var(axis=0)
    out = (feat - mu) / np.sqrt(var + EPS_BN) * gamma + beta
    return out.astype(np.float32)


def kernel(**inputs):
    cfg = dict(CONFIG)
    key = ("v2",)
    if key not in _CACHE:
        _CACHE[key] = _build(cfg)
    nc = _CACHE[key]
    in_maps = _host_prep(inputs, cfg)
    res = run_bass_kernel_spmd(
        nc, in_maps, core_ids=list(range(NCORES)), trace=cfg["trace"]
    )
    kernel.last_results = res
    v_parts = []
    for k in range(NCORES):
        x = res.results[k]["vmean"].reshape(2, B_LOC, 512)
        v_parts.append(x[0, :, 0:256] + x[1, :, 256:512])
    v_mean = np.concatenate(v_parts, axis=0)
    return _epilogue(v_mean, inputs)


# revision 27
# speedup vs baseline: 1.2325x; 1.0856x over previous
"""Trainium2 Bass kernel: Encoder_HieStackedCorr (UnCorrVmat_Detail, t_method='uncorr').

Math (per batch b):
    W1 = wn(U1_v, U1_g); W2 = wn(U2_v, U2_g)
    R = relu(V @ W1.T + b1)          [N, LR]
    L = relu(V @ W2.T + b2)          [N, LR]
    UnCorr = L @ R.T                 [N, N]
    d[n] = UnCorr[n, n] = sum_l L[n,l] R[n,l]
    dr = 1/sqrt(d + eps)
    A = 1 + I - dr dr^T * UnCorr
    v = mean_n (A @ V) = (1/N) * s @ V  where s[m] = N + 1 - dr[m] * (t . R[m,:]),
                                              t = sum_n dr[n] L[n,:]
    feat = v @ W_lin.T + b_lin ; out = batchnorm(feat)   (training-mode stats)

The N x N matrix is never materialized: mean-pooling commutes with the matmul,
collapsing the O(B N^2 (LR+D)) reference into O(B N D LR) work.

Kernel structure (v2):
  - W1 || W2 stacked into one [128, 128] lhsT: a single matmul pair per
    512-block produces both R (partitions 0-63) and L (64-127); one fused
    relu+bias on the whole [128, 512] PSUM tile.
  - Diag via ones[64,64] lhsT: d arrives already broadcast over 64
    partitions ([64, 512] PSUM), so rsqrt runs wide (ACT sqrt + DVE
    reciprocal_approx_fast) and no separate broadcast matmul is needed.
  - t via chained tensor_tensor_reduce (mult+add accumulate across blocks).
  - dr is folded into R (Rd = R * drrep on GpSimd), so the u-matmul yields
    c = dr*u directly and s = (N+1-c)/N comes from one ACT affine per block.
  - s scattered to column layout via a DRAM bounce, then 16 accumulating
    matmuls against natural-layout V give v_mean.

Sharding: data-parallel over batch, 4 batches per core on 8 cores.  The tiny
[32,256] linear + batchnorm epilogue (cross-core batch stats) runs on host.
"""

import os
import sys

import numpy as np

for _p in ("/opt/trn_rl_repo", "/root/.axon_site/_ro/trn_rl_repo"):
    if os.path.isdir(_p) and _p not in sys.path:
        sys.path.insert(0, _p)
        break

import ml_dtypes  # noqa: E402
import concourse.bass as bass  # noqa: E402
import concourse.bacc as bacc  # noqa: E402
import concourse.mybir as mybir  # noqa: E402
import concourse.tile as tile  # noqa: E402
from concourse.bass_utils import run_bass_kernel_spmd  # noqa: E402


def _ensure_ntff_hook():
    """Shim the missing ``antenv.axon_hooks`` registry so trace=True works."""
    import types

    try:
        from antenv.axon_hooks import get_axon_ntff_profile_hook  # noqa: F401
        return
    except ImportError:
        pass
    try:
        from trn_agent_boot.trn_boot import _ntff_profile_via_ctypes
        hook = _ntff_profile_via_ctypes("/opt/axon/libaxon_pjrt.so")
    except Exception:
        hook = None
    mod = types.ModuleType("antenv.axon_hooks")
    mod._hook = hook
    mod.get_axon_ntff_profile_hook = lambda: mod._hook
    mod.set_axon_ntff_profile_hook = lambda h: setattr(mod, "_hook", h)
    sys.modules["antenv.axon_hooks"] = mod


_ensure_ntff_hook()

# Problem constants (hardcoded; see module docstring).
B, N, D, LR, EMB = 32, 2048, 256, 64, 256
NCORES = 8
B_LOC = B // NCORES          # 4 batches per core
NT_B = N // 128              # 16 row-tiles per batch
NBLK = N // 512              # 4 512-col blocks per batch
VIN_W = 2 * N + NT_B * D     # per-batch packed width: vt (2*N) + v-nat (16*D)
EPS_DIAG = 1e-6
EPS_BN = 1e-5

F32 = mybir.dt.float32
BF16 = mybir.dt.bfloat16

CONFIG = dict(dt="bf16", trace=False)

_CACHE = {}


def _build(cfg):
    nc = bacc.Bacc("TRN2", target_bir_lowering=False, debug=False)

    vin_d = nc.dram_tensor("vin", [128, B_LOC, VIN_W], BF16, kind="ExternalInput").ap()
    w12_d = nc.dram_tensor("w12", [128, 2, 128], BF16, kind="ExternalInput").ap()
    b12_d = nc.dram_tensor("b12", [128, 1], F32, kind="ExternalInput").ap()
    out_d = nc.dram_tensor("vmean", [2, B_LOC * 512], F32, kind="ExternalOutput").ap()

    AF = mybir.ActivationFunctionType
    OP = mybir.AluOpType

    with tile.TileContext(nc) as tc:
        with (
            tc.tile_pool(name="const", bufs=1) as cpool,
            tc.tile_pool(name="vin", bufs=B_LOC) as vpool,
            tc.tile_pool(name="lr", bufs=3) as lrpool,
            tc.tile_pool(name="rd", bufs=3) as rdpool,
            tc.tile_pool(name="blk", bufs=3) as bpool,
            tc.tile_pool(name="sq", bufs=3) as sqpool,
            tc.tile_pool(name="dr", bufs=3) as drpool,
            tc.tile_pool(name="tsm", bufs=12) as tpool,
            tc.tile_pool(name="srow", bufs=3) as srpool,
            tc.tile_pool(name="scol", bufs=3) as scpool,
            tc.tile_pool(name="ps_lr", bufs=4, space="PSUM") as ps_lr,
            tc.tile_pool(name="ps_d", bufs=2, space="PSUM") as ps_d,
            tc.tile_pool(name="ps_u", bufs=1, space="PSUM") as ps_u,
            tc.tile_pool(name="ps_v", bufs=1, space="PSUM") as ps_v,
            tc.tile_pool(name="dram", bufs=4, space="DRAM") as dpool,
        ):
            # ---- constants / weights ----
            w12_sb = cpool.tile([128, 2 * 128], BF16)
            nc.sync.dma_start(
                w12_sb[:].rearrange("p (c m) -> p c m", c=2), w12_d[:]
            )
            b12_sb = cpool.tile([128, 1], F32)
            nc.sync.dma_start(b12_sb[:], b12_d[:])
            ones64 = cpool.tile([64, 64], BF16)
            nc.vector.memset(ones64[:], 1.0)
            eps64 = cpool.tile([64, 1], F32)
            nc.vector.memset(eps64[:], EPS_DIAG)
            out_sb = cpool.tile([2, B_LOC * 512], F32)

            # ---- per-batch inputs: vt halves first (feed main matmuls),
            # natural-V halves after (only needed by the final matmuls) ----
            vin = {}
            for b in range(B_LOC):
                vin[b] = vpool.tile([128, VIN_W], BF16, tag="vin", name=f"vin{b}")
                nc.sync.dma_start(vin[b][:, 0:2 * N], vin_d[:, b, 0:2 * N])
            for b in range(B_LOC):
                nc.sync.dma_start(vin[b][:, 2 * N:], vin_d[:, b, 2 * N:])

            # ---- PE warm-up: ~5us of dummy matmuls during the input-DMA
            # dead time.  The HAM clock gate only lifts (1.2 -> 2.4 GHz)
            # after ~3.4us of SUSTAINED matmul activity; without this the
            # whole kernel runs at half PE clock. ----
            junkw = cpool.tile([128, 128], BF16)
            nc.vector.memset(junkw[:], 1.0)
            junk = cpool.tile([128, 512], BF16)
            nc.vector.memset(junk[:], 1.0)
            for w in range(14):
                w_ps = ps_lr.tile([128, 512], F32, tag="lrps")
                nc.tensor.matmul(
                    w_ps[:], junkw[:], junk[:], start=True, stop=True,
                )

            # Per-batch state carried between the emission helpers below.
            st = {}

            def phase_a_mains(b):
                """All 8 main matmuls of a batch, back-to-back: a ~3.4us
                contiguous PE burst that (re)lifts the HAM clock gate."""
                s = st[b]
                vt_v = vin[b][:, 0:2 * N]
                s["lr_ps"] = []
                for blk in range(NBLK):
                    f0 = blk * 512
                    lr_ps = ps_lr.tile([128, 512], F32, tag="lrps")
                    s["lr_ps"].append(lr_ps)
                    for c in range(2):
                        nc.tensor.matmul(
                            lr_ps[:],
                            w12_sb[:, c * 128:(c + 1) * 128],
                            vt_v[:, c * N + f0:c * N + f0 + 512],
                            start=(c == 0), stop=(c == 1),
                        )

            def phase_a_block(b, blk):
                """Elementwise chain + small matmuls for one 512-col block."""
                s = st[b]
                f0 = blk * 512
                lr_ps = s["lr_ps"][blk]
                # split relu (cross-base PSUM read is legal); base 0 outputs.
                nc.scalar.activation(
                    s["R"][:, f0:f0 + 512], lr_ps[0:64, :], AF.Relu,
                    bias=b12_sb[0:64, :], scale=1.0,
                )
                if blk % 2 == 0:
                    nc.scalar.activation(
                        s["L"][:, f0:f0 + 512], lr_ps[64:128, :], AF.Relu,
                        bias=b12_sb[64:128, :], scale=1.0,
                    )
                else:
                    nc.vector.tensor_scalar(
                        s["L"][:, f0:f0 + 512], lr_ps[64:128, :],
                        b12_sb[64:128, :], 0.0, OP.add, OP.max,
                    )
                # prod = L * R on DVE (block-chain critical: feeds diag)
                prod = bpool.tile([64, 512], BF16, tag="prod")
                nc.vector.tensor_tensor(
                    prod[:], s["R"][:, f0:f0 + 512],
                    s["L"][:, f0:f0 + 512], OP.mult,
                )
                # d broadcast over 64 partitions via ones-lhsT matmul
                d_ps = ps_d.tile([64, 512], F32, tag="dps")
                nc.tensor.matmul(
                    d_ps[:], ones64[:], prod[:], start=True, stop=True,
                )
                # drrep = 1/sqrt(d + eps), computed wide
                sqrep = sqpool.tile([64, 512], F32, tag="sq")
                nc.scalar.activation(
                    sqrep[:], d_ps[:], AF.Sqrt, bias=eps64[:], scale=1.0,
                )
                nc.vector.reciprocal_approx_fast(
                    s["drrep"][:, f0:f0 + 512], sqrep[:]
                )
                # ldr = L * drrep (summed for t); Rd = R * drrep so the
                # u-matmul yields c = dr*u directly.  Per-block ops keep the
                # t chain latency low; GpSimd takes them (off ACT/DVE).
                nc.gpsimd.tensor_tensor(
                    s["ldr"][:, f0:f0 + 512], s["L"][:, f0:f0 + 512],
                    s["drrep"][:, f0:f0 + 512], OP.mult,
                )
                nc.gpsimd.tensor_tensor(
                    s["Rd"][:, f0:f0 + 512], s["R"][:, f0:f0 + 512],
                    s["drrep"][:, f0:f0 + 512], OP.mult,
                )


            def phase_b_early(b):
                """t, u-matmuls, s rows, and the DRAM scatter bounces."""
                s = st[b]
                t_f = tpool.tile([64, 1], F32, tag="tf", name=f"tf{b}")
                nc.vector.tensor_reduce(
                    t_f[:], s["ldr"][:], mybir.AxisListType.X, OP.add,
                )
                t_bf = tpool.tile([64, 1], BF16, tag="tbf", name=f"tbf{b}")
                nc.scalar.activation(t_bf[:], t_f[:], AF.Copy)
                s_row = srpool.tile([1, N], BF16, tag="srow")
                s["s_col"] = scpool.tile([128, NT_B], BF16, tag="scol", name=f"scol{b}")
                for blk in range(NBLK):
                    f0 = blk * 512
                    u_ps = ps_u.tile([1, 512], F32, tag="ups")
                    nc.tensor.matmul(
                        u_ps[:], t_bf[:], s["Rd"][:, f0:f0 + 512],
                        start=True, stop=True,
                    )
                    if blk % 2 == 0:
                        nc.scalar.activation(
                            s_row[:, f0:f0 + 512], u_ps[:], AF.Copy,
                            bias=float(N + 1) / N, scale=-1.0 / N,
                        )
                    else:
                        nc.vector.tensor_scalar(
                            s_row[:, f0:f0 + 512], u_ps[:],
                            -1.0 / N, float(N + 1) / N, OP.mult, OP.add,
                        )
                # partition scatter: DRAM bounce out, then one xbar
                # transpose-DMA back ([16,128] -> [128,16] at line rate).
                s_dram = dpool.tile([1, N], BF16, tag="sdram", name=f"sdram{b}")
                nc.sync.dma_start(s_dram[:], s_row[:])
                nc.sync.dma_start_transpose(
                    s["s_col"][:], s_dram.rearrange("a (j p) -> (a j) p", p=128)
                )

            def phase_b_final(b):
                """v_mean = s^T @ V, then output.  Row-tile PAIRS share one
                [2,512] matmul (same total rows, half the instructions):
                row 0 cols 0:256 and row 1 cols 256:512 hold the two useful
                quadrants; one cross-partition PSUM add combines them."""
                s = st[b]
                vnat = vin[b][:, 2 * N:]
                v_ps = ps_v.tile([2, 512], F32, tag="vps")
                for jp in range(NT_B // 2):
                    j = 2 * jp
                    nc.tensor.matmul(
                        v_ps[:], s["s_col"][:, j:j + 2],
                        vnat[:, j * D:(j + 2) * D],
                        start=(jp == 0), stop=(jp == NT_B // 2 - 1),
                    )
                nc.scalar.activation(
                    out_sb[:, b * 512:(b + 1) * 512], v_ps[:], AF.Copy,
                )
                nc.sync.dma_start(
                    out_d[:, b * 512:(b + 1) * 512],
                    out_sb[:, b * 512:(b + 1) * 512],
                )

            def new_batch(b):
                st[b] = {
                    "R": lrpool.tile([64, N], BF16, tag="R", name=f"R{b}"),
                    "L": lrpool.tile([64, N], BF16, tag="L", name=f"L{b}"),
                    "Rd": rdpool.tile([64, N], BF16, tag="Rd", name=f"Rd{b}"),
                    "ldr": rdpool.tile([64, N], BF16, tag="ldr", name=f"ldr{b}"),
                    "drrep": drpool.tile([64, N], F32, tag="dr", name=f"dr{b}"),
                }

            # Software-pipelined emission at depth 2: batch b-1's u/s phase
            # and batch b-2's final matmuls queue BEHIND batch b's ready
            # main matmuls, so DMA/t-gated work never head-of-line-blocks
            # the in-order PE queue.
            for b in range(B_LOC):
                new_batch(b)
                phase_a_mains(b)
                phase_a_block(b, 0)
                phase_a_block(b, 1)
                if b >= 2:
                    phase_b_final(b - 2)
                phase_a_block(b, 2)
                phase_a_block(b, 3)
                if b >= 1:
                    phase_b_early(b - 1)
            phase_b_early(B_LOC - 1)
            phase_b_final(B_LOC - 2)
            phase_b_final(B_LOC - 1)

    nc.compile()
    return nc


def _host_prep(inputs, cfg):
    """Weight-norm, packing, casts; returns per-core input maps."""
    def wn(v, g):
        return v * (g / np.linalg.norm(v.astype(np.float64), axis=1)).astype(
            np.float32
        )[:, None]

    W1 = wn(np.asarray(inputs["U1_v"], np.float32), np.asarray(inputs["U1_g"], np.float32))
    W2 = wn(np.asarray(inputs["U2_v"], np.float32), np.asarray(inputs["U2_g"], np.float32))
    # lhsT layout [d, m]: m 0-63 -> R (W1), 64-127 -> L (W2); split d in 2 chunks
    W12T = np.concatenate([W1.T, W2.T], axis=1)          # [D, 128]
    w12 = np.ascontiguousarray(
        W12T.reshape(2, 128, 128)
    ).astype(ml_dtypes.bfloat16)                          # [c, d, m]
    w12 = np.ascontiguousarray(w12.transpose(1, 0, 2))    # [d, c, m]
    b12 = np.concatenate([
        np.asarray(inputs["U1_b"], np.float32),
        np.asarray(inputs["U2_b"], np.float32),
    ]).reshape(128, 1)

    V = np.asarray(inputs["Vmat"], np.float32)  # [B, N, D]
    Vb = V.astype(ml_dtypes.bfloat16)
    in_maps = []
    for k in range(NCORES):
        packs = []
        for b in range(B_LOC):
            Vk = Vb[k * B_LOC + b]                                    # [N, D]
            vt = Vk.T.reshape(2, 128, N).transpose(1, 0, 2).reshape(128, 2 * N)
            vn = Vk.reshape(NT_B, 128, D).transpose(1, 0, 2).reshape(128, NT_B * D)
            packs.append(np.concatenate([vt, vn], axis=1))            # [128, VIN_W]
        vin = np.ascontiguousarray(np.stack(packs, axis=1))           # [128, B_LOC, VIN_W]
        in_maps.append({"vin": vin, "w12": w12, "b12": b12})
    return in_maps


def _epilogue(v_mean, inputs):
    """feat = v_mean @ W_lin.T + b_lin, then training-mode batchnorm."""
    W_lin = np.asarray(inputs["W_lin"], np.float32)
    b_lin = np.asarray(inputs["b_lin"], np.float32)
    gamma = np.asarray(inputs["gamma"], np.float32)
    beta = np.asarray(inputs["beta"], np.float32)
    feat = v_mean.astype(np.float32) @ W_lin.T + b_lin
    mu = feat.mean(axis=0)
    var = feat.var(axis=0)
    out = (feat - mu) / np.sqrt(var + EPS_BN) * gamma + beta
    return out.astype(np.float32)


def kernel(**inputs):
    cfg = dict(CONFIG)
    key = ("v2",)
    if key not in _CACHE:
        _CACHE[key] = _build(cfg)
    nc = _CACHE[key]
    in_maps = _host_prep(inputs, cfg)
    res = run_bass_kernel_spmd(
        nc, in_maps, core_ids=list(range(NCORES)), trace=cfg["trace"]
    )
    kernel.last_results = res
    v_parts = []
    for k in range(NCORES):
        x = res.results[k]["vmean"].reshape(2, B_LOC, 512)
        v_parts.append(x[0, :, 0:256] + x[1, :, 256:512])
    v_mean = np.concatenate(v_parts, axis=0)
    return _epilogue(v_mean, inputs)


# revision 28
# speedup vs baseline: 1.3139x; 1.0661x over previous
"""Trainium2 Bass kernel: Encoder_HieStackedCorr (UnCorrVmat_Detail, t_method='uncorr').

Math (per batch b):
    W1 = wn(U1_v, U1_g); W2 = wn(U2_v, U2_g)
    R = relu(V @ W1.T + b1)          [N, LR]
    L = relu(V @ W2.T + b2)          [N, LR]
    UnCorr = L @ R.T                 [N, N]
    d[n] = UnCorr[n, n] = sum_l L[n,l] R[n,l]
    dr = 1/sqrt(d + eps)
    A = 1 + I - dr dr^T * UnCorr
    v = mean_n (A @ V) = (1/N) * s @ V  where s[m] = N + 1 - dr[m] * (t . R[m,:]),
                                              t = sum_n dr[n] L[n,:]
    feat = v @ W_lin.T + b_lin ; out = batchnorm(feat)   (training-mode stats)

The N x N matrix is never materialized: mean-pooling commutes with the matmul,
collapsing the O(B N^2 (LR+D)) reference into O(B N D LR) work.

Kernel structure (v2):
  - W1 || W2 stacked into one [128, 128] lhsT: a single matmul pair per
    512-block produces both R (partitions 0-63) and L (64-127); one fused
    relu+bias on the whole [128, 512] PSUM tile.
  - Diag via ones[64,64] lhsT: d arrives already broadcast over 64
    partitions ([64, 512] PSUM), so rsqrt runs wide (ACT sqrt + DVE
    reciprocal_approx_fast) and no separate broadcast matmul is needed.
  - t via chained tensor_tensor_reduce (mult+add accumulate across blocks).
  - dr is folded into R (Rd = R * drrep on GpSimd), so the u-matmul yields
    c = dr*u directly and s = (N+1-c)/N comes from one ACT affine per block.
  - s scattered to column layout via a DRAM bounce, then 16 accumulating
    matmuls against natural-layout V give v_mean.

Sharding: data-parallel over batch, 4 batches per core on 8 cores.  The tiny
[32,256] linear + batchnorm epilogue (cross-core batch stats) runs on host.
"""

import os
import sys

import numpy as np

for _p in ("/opt/trn_rl_repo", "/root/.axon_site/_ro/trn_rl_repo"):
    if os.path.isdir(_p) and _p not in sys.path:
        sys.path.insert(0, _p)
        break

import ml_dtypes  # noqa: E402
import concourse.bass as bass  # noqa: E402
import concourse.bacc as bacc  # noqa: E402
import concourse.mybir as mybir  # noqa: E402
import concourse.tile as tile  # noqa: E402
from concourse.bass_utils import run_bass_kernel_spmd  # noqa: E402


def _ensure_ntff_hook():
    """Shim the missing ``antenv.axon_hooks`` registry so trace=True works."""
    import types

    try:
        from antenv.axon_hooks import get_axon_ntff_profile_hook  # noqa: F401
        return
    except ImportError:
        pass
    try:
        from trn_agent_boot.trn_boot import _ntff_profile_via_ctypes
        hook = _ntff_profile_via_ctypes("/opt/axon/libaxon_pjrt.so")
    except Exception:
        hook = None
    mod = types.ModuleType("antenv.axon_hooks")
    mod._hook = hook
    mod.get_axon_ntff_profile_hook = lambda: mod._hook
    mod.set_axon_ntff_profile_hook = lambda h: setattr(mod, "_hook", h)
    sys.modules["antenv.axon_hooks"] = mod


_ensure_ntff_hook()

# Problem constants (hardcoded; see module docstring).
B, N, D, LR, EMB = 32, 2048, 256, 64, 256
NCORES = 8
B_LOC = B // NCORES          # 4 batches per core
NT_B = N // 128              # 16 row-tiles per batch
NBLK = N // 512              # 4 512-col blocks per batch
VIN_W = 2 * N + NT_B * D     # per-batch packed width: vt (2*N) + v-nat (16*D)
EPS_DIAG = 1e-6
EPS_BN = 1e-5

F32 = mybir.dt.float32
BF16 = mybir.dt.bfloat16

CONFIG = dict(dt="bf16", trace=False)

_CACHE = {}


def _build(cfg):
    nc = bacc.Bacc("TRN2", target_bir_lowering=False, debug=False)

    vin_d = nc.dram_tensor("vin", [128, B_LOC, VIN_W], BF16, kind="ExternalInput").ap()
    w12_d = nc.dram_tensor("w12", [128, 2, 128], BF16, kind="ExternalInput").ap()
    b12_d = nc.dram_tensor("b12", [128, 1], F32, kind="ExternalInput").ap()
    out_d = nc.dram_tensor("vmean", [2, B_LOC * 512], F32, kind="ExternalOutput").ap()

    AF = mybir.ActivationFunctionType
    OP = mybir.AluOpType

    with tile.TileContext(nc) as tc:
        with (
            tc.tile_pool(name="const", bufs=1) as cpool,
            tc.tile_pool(name="vin", bufs=B_LOC) as vpool,
            tc.tile_pool(name="lr", bufs=3) as lrpool,
            tc.tile_pool(name="rd", bufs=3) as rdpool,
            tc.tile_pool(name="blk", bufs=3) as bpool,
            tc.tile_pool(name="sq", bufs=3) as sqpool,
            tc.tile_pool(name="dr", bufs=3) as drpool,
            tc.tile_pool(name="tsm", bufs=12) as tpool,
            tc.tile_pool(name="srow", bufs=3) as srpool,
            tc.tile_pool(name="scol", bufs=3) as scpool,
            tc.tile_pool(name="ps_lr", bufs=4, space="PSUM") as ps_lr,
            tc.tile_pool(name="ps_d", bufs=2, space="PSUM") as ps_d,
            tc.tile_pool(name="ps_u", bufs=1, space="PSUM") as ps_u,
            tc.tile_pool(name="ps_v", bufs=1, space="PSUM") as ps_v,
            tc.tile_pool(name="dram", bufs=4, space="DRAM") as dpool,
        ):
            # ---- constants / weights ----
            w12_sb = cpool.tile([128, 2 * 128], BF16)
            nc.sync.dma_start(
                w12_sb[:].rearrange("p (c m) -> p c m", c=2), w12_d[:]
            )
            b12_sb = cpool.tile([128, 1], F32)
            nc.sync.dma_start(b12_sb[:], b12_d[:])
            ones64 = cpool.tile([64, 64], BF16)
            nc.vector.memset(ones64[:], 1.0)
            eps64 = cpool.tile([64, 1], F32)
            nc.vector.memset(eps64[:], EPS_DIAG)
            out_sb = cpool.tile([2, B_LOC * 512], F32)

            # ---- per-batch inputs: vt halves first (feed main matmuls),
            # natural-V halves after (only needed by the final matmuls) ----
            vin = {}
            for b in range(B_LOC):
                vin[b] = vpool.tile([128, VIN_W], BF16, tag="vin", name=f"vin{b}")
                nc.sync.dma_start(vin[b][:, 0:2 * N], vin_d[:, b, 0:2 * N])
            for b in range(B_LOC):
                nc.sync.dma_start(vin[b][:, 2 * N:], vin_d[:, b, 2 * N:])

            # ---- PE warm-up: ~5us of dummy matmuls during the input-DMA
            # dead time.  The HAM clock gate only lifts (1.2 -> 2.4 GHz)
            # after ~3.4us of SUSTAINED matmul activity; without this the
            # whole kernel runs at half PE clock. ----
            junkw = cpool.tile([128, 128], BF16)
            nc.vector.memset(junkw[:], 1.0)
            junk = cpool.tile([128, 512], BF16)
            nc.vector.memset(junk[:], 1.0)
            for w in range(14):
                w_ps = ps_lr.tile([128, 512], F32, tag="lrps")
                nc.tensor.matmul(
                    w_ps[:], junkw[:], junk[:], start=True, stop=True,
                )

            # Per-batch state carried between the emission helpers below.
            st = {}

            def phase_a_mains(b):
                """All 8 main matmuls of a batch, back-to-back: a ~3.4us
                contiguous PE burst that (re)lifts the HAM clock gate."""
                s = st[b]
                vt_v = vin[b][:, 0:2 * N]
                s["lr_ps"] = []
                for blk in range(NBLK):
                    f0 = blk * 512
                    lr_ps = ps_lr.tile([128, 512], F32, tag="lrps")
                    s["lr_ps"].append(lr_ps)
                    for c in range(2):
                        nc.tensor.matmul(
                            lr_ps[:],
                            w12_sb[:, c * 128:(c + 1) * 128],
                            vt_v[:, c * N + f0:c * N + f0 + 512],
                            start=(c == 0), stop=(c == 1),
                        )

            def phase_a_block(b, blk):
                """Elementwise chain + small matmuls for one 512-col block."""
                s = st[b]
                f0 = blk * 512
                lr_ps = s["lr_ps"][blk]
                # split relu (cross-base PSUM read is legal); base 0 outputs.
                nc.scalar.activation(
                    s["R"][:, f0:f0 + 512], lr_ps[0:64, :], AF.Relu,
                    bias=b12_sb[0:64, :], scale=1.0,
                )
                if blk % 2 == 0:
                    nc.scalar.activation(
                        s["L"][:, f0:f0 + 512], lr_ps[64:128, :], AF.Relu,
                        bias=b12_sb[64:128, :], scale=1.0,
                    )
                else:
                    nc.vector.tensor_scalar(
                        s["L"][:, f0:f0 + 512], lr_ps[64:128, :],
                        b12_sb[64:128, :], 0.0, OP.add, OP.max,
                    )
                # prod = L * R on DVE (block-chain critical: feeds diag)
                prod = bpool.tile([64, 512], BF16, tag="prod")
                nc.vector.tensor_tensor(
                    prod[:], s["R"][:, f0:f0 + 512],
                    s["L"][:, f0:f0 + 512], OP.mult,
                )
                # d broadcast over 64 partitions via ones-lhsT matmul
                d_ps = ps_d.tile([64, 512], F32, tag="dps")
                nc.tensor.matmul(
                    d_ps[:], ones64[:], prod[:], start=True, stop=True,
                )
                # drrep = 1/sqrt(d + eps), computed wide
                sqrep = sqpool.tile([64, 512], F32, tag="sq")
                nc.scalar.activation(
                    sqrep[:], d_ps[:], AF.Sqrt, bias=eps64[:], scale=1.0,
                )
                nc.vector.reciprocal_approx_fast(
                    s["drrep"][:, f0:f0 + 512], sqrep[:]
                )
                # ldr = L * drrep (summed for t); Rd = R * drrep so the
                # u-matmul yields c = dr*u directly.  Per-block ops keep the
                # t chain latency low; GpSimd takes them (off ACT/DVE).
                nc.gpsimd.tensor_tensor(
                    s["ldr"][:, f0:f0 + 512], s["L"][:, f0:f0 + 512],
                    s["drrep"][:, f0:f0 + 512], OP.mult,
                )
                nc.gpsimd.tensor_tensor(
                    s["Rd"][:, f0:f0 + 512], s["R"][:, f0:f0 + 512],
                    s["drrep"][:, f0:f0 + 512], OP.mult,
                )


            def phase_b_early(b):
                """t, u-matmuls, s rows, and the DRAM scatter bounces."""
                s = st[b]
                t_f = tpool.tile([64, 1], F32, tag="tf", name=f"tf{b}")
                nc.vector.tensor_reduce(
                    t_f[:], s["ldr"][:], mybir.AxisListType.X, OP.add,
                )
                t_bf = tpool.tile([64, 1], BF16, tag="tbf", name=f"tbf{b}")
                nc.scalar.activation(t_bf[:], t_f[:], AF.Copy)
                s_row = srpool.tile([1, N], BF16, tag="srow")
                s["s_col"] = scpool.tile([128, NT_B], BF16, tag="scol", name=f"scol{b}")
                for blk in range(NBLK):
                    f0 = blk * 512
                    u_ps = ps_u.tile([1, 512], F32, tag="ups")
                    nc.tensor.matmul(
                        u_ps[:], t_bf[:], s["Rd"][:, f0:f0 + 512],
                        start=True, stop=True,
                    )
                    if blk % 2 == 0:
                        nc.scalar.activation(
                            s_row[:, f0:f0 + 512], u_ps[:], AF.Copy,
                            bias=float(N + 1) / N, scale=-1.0 / N,
                        )
                    else:
                        nc.vector.tensor_scalar(
                            s_row[:, f0:f0 + 512], u_ps[:],
                            -1.0 / N, float(N + 1) / N, OP.mult, OP.add,
                        )
                # partition scatter: DRAM bounce out, then one xbar
                # transpose-DMA back ([16,128] -> [128,16] at line rate).
                s_dram = dpool.tile([1, N], BF16, tag="sdram", name=f"sdram{b}")
                nc.sync.dma_start(s_dram[:], s_row[:])
                nc.sync.dma_start_transpose(
                    s["s_col"][:], s_dram.rearrange("a (j p) -> (a j) p", p=128)
                )

            def phase_b_final(b):
                """v_mean = s^T @ V, then output.  Row-tile PAIRS share one
                [2,512] matmul (same total rows, half the instructions):
                row 0 cols 0:256 and row 1 cols 256:512 hold the two useful
                quadrants; one cross-partition PSUM add combines them."""
                s = st[b]
                vnat = vin[b][:, 2 * N:]
                v_ps = ps_v.tile([2, 512], F32, tag="vps")
                for jp in range(NT_B // 2):
                    j = 2 * jp
                    nc.tensor.matmul(
                        v_ps[:], s["s_col"][:, j:j + 2],
                        vnat[:, j * D:(j + 2) * D],
                        start=(jp == 0), stop=(jp == NT_B // 2 - 1),
                    )
                nc.scalar.activation(
                    out_sb[:, b * 512:(b + 1) * 512], v_ps[:], AF.Copy,
                )
                nc.sync.dma_start(
                    out_d[:, b * 512:(b + 1) * 512],
                    out_sb[:, b * 512:(b + 1) * 512],
                )

            def new_batch(b):
                st[b] = {
                    "R": lrpool.tile([64, N], BF16, tag="R", name=f"R{b}"),
                    "L": lrpool.tile([64, N], BF16, tag="L", name=f"L{b}"),
                    "Rd": rdpool.tile([64, N], BF16, tag="Rd", name=f"Rd{b}"),
                    "ldr": rdpool.tile([64, N], BF16, tag="ldr", name=f"ldr{b}"),
                    "drrep": drpool.tile([64, N], F32, tag="dr", name=f"dr{b}"),
                }

            # Software-pipelined emission at depth 2: batch b-1's u/s phase
            # and batch b-2's final matmuls queue BEHIND batch b's ready
            # main matmuls, so DMA/t-gated work never head-of-line-blocks
            # the in-order PE queue.
            for b in range(B_LOC):
                new_batch(b)
                phase_a_mains(b)
                phase_a_block(b, 0)
                phase_a_block(b, 1)
                if b >= 1:
                    phase_b_early(b - 1)
                if b >= 2:
                    phase_b_final(b - 2)
                phase_a_block(b, 2)
                phase_a_block(b, 3)
            phase_b_early(B_LOC - 1)
            phase_b_final(B_LOC - 2)
            phase_b_final(B_LOC - 1)

    nc.compile()
    return nc


def _host_prep(inputs, cfg):
    """Weight-norm, packing, casts; returns per-core input maps."""
    def wn(v, g):
        return v * (g / np.linalg.norm(v.astype(np.float64), axis=1)).astype(
            np.float32
        )[:, None]

    W1 = wn(np.asarray(inputs["U1_v"], np.float32), np.asarray(inputs["U1_g"], np.float32))
    W2 = wn(np.asarray(inputs["U2_v"], np.float32), np.asarray(inputs["U2_g"], np.float32))
    # lhsT layout [d, m]: m 0-63 -> R (W1), 64-127 -> L (W2); split d in 2 chunks
    W12T = np.concatenate([W1.T, W2.T], axis=1)          # [D, 128]
    w12 = np.ascontiguousarray(
        W12T.reshape(2, 128, 128)
    ).astype(ml_dtypes.bfloat16)                          # [c, d, m]
    w12 = np.ascontiguousarray(w12.transpose(1, 0, 2))    # [d, c, m]
    b12 = np.concatenate([
        np.asarray(inputs["U1_b"], np.float32),
        np.asarray(inputs["U2_b"], np.float32),
    ]).reshape(128, 1)

    V = np.asarray(inputs["Vmat"], np.float32)  # [B, N, D]
    Vb = V.astype(ml_dtypes.bfloat16)
    in_maps = []
    for k in range(NCORES):
        packs = []
        for b in range(B_LOC):
            Vk = Vb[k * B_LOC + b]                                    # [N, D]
            vt = Vk.T.reshape(2, 128, N).transpose(1, 0, 2).reshape(128, 2 * N)
            vn = Vk.reshape(NT_B, 128, D).transpose(1, 0, 2).reshape(128, NT_B * D)
            packs.append(np.concatenate([vt, vn], axis=1))            # [128, VIN_W]
        vin = np.ascontiguousarray(np.stack(packs, axis=1))           # [128, B_LOC, VIN_W]
        in_maps.append({"vin": vin, "w12": w12, "b12": b12})
    return in_maps


def _epilogue(v_mean, inputs):
    """feat = v_mean @ W_lin.T + b_lin, then training-mode batchnorm."""
    W_lin = np.asarray(inputs["W_lin"], np.float32)
    b_lin = np.asarray(inputs["b_lin"], np.float32)
    gamma = np.asarray(inputs["gamma"], np.float32)
    beta = np.asarray(inputs["beta"], np.float32)
    feat = v_mean.astype(np.float32) @ W_lin.T + b_lin
    mu = feat.mean(axis=0)
    var = feat.var(axis=0)
    out = (feat - mu) / np.sqrt(var + EPS_BN) * gamma + beta
    return out.astype(np.float32)


def kernel(**inputs):
    cfg = dict(CONFIG)
    key = ("v2",)
    if key not in _CACHE:
        _CACHE[key] = _build(cfg)
    nc = _CACHE[key]
    in_maps = _host_prep(inputs, cfg)
    res = run_bass_kernel_spmd(
        nc, in_maps, core_ids=list(range(NCORES)), trace=cfg["trace"]
    )
    kernel.last_results = res
    v_parts = []
    for k in range(NCORES):
        x = res.results[k]["vmean"].reshape(2, B_LOC, 512)
        v_parts.append(x[0, :, 0:256] + x[1, :, 256:512])
    v_mean = np.concatenate(v_parts, axis=0)
    return _epilogue(v_mean, inputs)
